# revision 40
# baseline (speedup 1.0000x reference)
"""VQ codebook kernel (nn_CP_34041910788864) for 8 Trainium2 NeuronCores.

Reference computation:
  flat = IP_score.reshape(13056, 512)
  d[n,k] = ||flat_n||^2 + ||emb_k||^2 - 2 flat_n . emb_k      (K=2048)
  top3 -> nearest idx1, third idx3
  feature_EMA = emb[idx1] (numerically x + sg(q - x) == q up to 1e-7)
  k_loss = 0.25 * mean((emb[idx1] - x)^2) = 0.25 * sum_n d[n,idx1] / (N*D)
  cp_score = 1 - sqrt(sum_n d[n,idx1]) / sqrt(sum_n d[n,idx3])

Sharding: data-parallel over rows. Rows padded 13056 -> 13312 = 8 * 1664,
each core takes 1664 rows (13 tiles of 128). The codebook is replicated.

Device per core:
  - transpose emb to [d, k] layout (TensorE transpose), scale 2, round to
    fp32r (12-bit mantissa). The nearest-neighbour margins on this data are
    ~0.003 while fp32r scoring noise is ~1e-5 (the rounding is absorbed
    exactly into the score as s = 2*round(x).round(e) - ||e||^2, computed
    identically here and when validating), verified flip-free vs f64.
  - embnorm[k] in full fp32 via ACT square+accum; folded into the matmul
    accumulation as a 2-row bias (hi/lo fp32r split of -||e||^2, exact to
    ~1.5e-5) so no vector-engine subtract is needed.
  - per row tile: PSUM[r,k] = 2*x.e - ||e||^2 via 5 fp32r matmuls per
    512-wide chunk; DVE max/max_index read PSUM directly; nearest rows
    gathered from emb by indirect DMA (row-local gather).
  - host: exact f64 reduction of the two scalar outputs
    (sum d1 = ||flat||^2_F - sum v1 where v1 = max_k s)
"""

import numpy as np

N_CORES = 8
N_ROWS = 13056          # 256*51
R_PAD = 13312           # 8 * 1664
R_CORE = 1664           # rows per core
T_TILES = 13            # 13 * 128 = 1664
K = 2048
D = 512
P = 128
COMMITMENT_COST = 0.25

_cache = {}


def _build_program():
    import concourse.bacc as bacc
    import concourse.bass as bass
    import concourse.mybir as mybir
    import concourse.tile as tile
    from concourse.masks import make_identity

    f32 = mybir.dt.float32
    f32r = mybir.dt.float32r
    u32 = mybir.dt.uint32

    nc = bacc.Bacc("TRN2", target_bir_lowering=False)
    # x5[p, t, c, i] = x[t*128+i, c*128+p]: per (p, t) the [4, 128] block is
    # contiguous, so each x-tile load is 128 descriptors of 2KB
    xT = nc.declare_dram_parameter("xT", [P, T_TILES, 4, P], f32,
                                   isOutput=False)
    emb = nc.declare_dram_parameter("emb", [K, D], f32, isOutput=False)
    feat = nc.declare_dram_parameter("feat", [R_CORE, D], f32, isOutput=True)
    vals = nc.declare_dram_parameter("vals", [P, T_TILES, 8], f32, isOutput=True)
    idxs = nc.declare_dram_parameter("idxs", [P, T_TILES, 8], u32, isOutput=True)
    en_dram = nc.dram_tensor("en_scratch", [K], f32)
    eb_dram = nc.dram_tensor("eb_scratch", [2, K], f32)

    ACT_COPY = mybir.ActivationFunctionType.Copy
    ACT_SQUARE = mybir.ActivationFunctionType.Square

    with tile.TileContext(nc) as tc:
        with tc.tile_pool(name="const", bufs=1) as cp, \
             tc.tile_pool(name="work", bufs=3) as wk:

            def load_x(t):
                xt = wk.tile([P, 4, P], f32, tag="xt", name=f"xt{t}")
                nc.sync.dma_start(out=xt[:], in_=xT.ap()[:, t])
                xr = wk.tile([P, 4, P], f32r, tag="xr", name=f"xr{t}")
                nc.scalar.activation(xr[:], xt[:], ACT_COPY, scale=2.0)
                return xr

            # ---------------- setup ----------------
            with tc.tile_pool(name="setup", bufs=1) as su, \
                 tc.tile_pool(name="pss", bufs=2, space="PSUM") as pss:

                ident = su.tile([P, P], f32, tag="ident")
                make_identity(nc, ident[:])
                # bias lhsT rows are -1 so ebias can hold +embnorm hi/lo
                # (saves a negation pass on the critical setup chain)
                ones_f = su.tile([2, P], f32, tag="ones_f")
                nc.vector.memset(ones_f[:], -1.0)
                ones2 = cp.tile([2, P], f32r, tag="ones2")
                nc.vector.tensor_copy(ones2[:], ones_f[:])

                # prologue: first tiles' x loads + rounding ahead of the
                # heavy setup so tile 0 can start as soon as the first
                # codebook quarter is transposed
                xr_pre = {t: load_x(t) for t in range(2)}

                # natural-layout codebook [128, 16, 512], loaded in 4 chunks
                # so transposes/norms pipeline with the load
                embn = su.tile([P, 16, D], f32, tag="embn")
                emb_v = emb.ap().rearrange("(kb p) d -> p kb d", p=P)
                # one fp32r [d, k]-layout tile per 512-wide k quarter, so the
                # main loop's quarter-q matmuls only wait on their own quarter
                embT2rq = [
                    cp.tile([P, 4, 512], f32r, tag=f"embT2r{q}",
                            name=f"embT2r{q}")
                    for q in range(4)
                ]
                en_col = su.tile([P, 16], f32, tag="en_col")
                b12c = su.tile([P, 2, 16], f32r, tag="b12c")
                t2c = su.tile([P, 16], f32, tag="t2c")
                eb_v = eb_dram.ap().rearrange("two (kb p) -> p two kb", p=P)
                ebias_q = [
                    cp.tile([2, 512], f32r, tag=f"ebias{q}", name=f"ebias{q}")
                    for q in range(4)
                ]
                for g in range(8):
                    nc.sync.dma_start(
                        out=embn[:, 2 * g:2 * (g + 1)],
                        in_=emb_v[:, 2 * g:2 * (g + 1)],
                    )
                for kb in range(16):
                    # embT2r[p, c, k] = round12(2 * emb[k, c*128+p])
                    # copies alternate ACT/DVE to halve the serial setup span
                    psT = pss.tile([P, 4, P], f32, tag="psT")
                    for c in range(4):
                        nc.tensor.transpose(
                            psT[:, c], embn[:, kb, c * P:(c + 1) * P], ident[:]
                        )
                    # the 2x scale lives on the x side (xr = round12(2x)),
                    # so these are plain rounding copies
                    dst = embT2rq[kb // 4][:, :, (kb % 4) * P:(kb % 4 + 1) * P]
                    nc.scalar.activation(dst, psT[:], ACT_COPY)
                    # embnorm: en_col[p, kb] = sum_d emb[kb*128+p, d]^2 (fp32)
                    sq_scr = wk.tile([P, D], f32, tag="sq_scr",
                                     name=f"sq_scr{kb}")
                    nc.scalar.activation(
                        sq_scr[:], embn[:, kb], ACT_SQUARE,
                        accum_out=en_col[:, kb:kb + 1],
                    )
                    if kb % 4 == 3:
                        # ebias hi/lo split (+embnorm; sign carried by ones2)
                        # per 4-kb group, bounced out as soon as available
                        g = kb // 4
                        gs = slice(4 * g, 4 * (g + 1))
                        nc.vector.tensor_copy(b12c[:, 0, gs], en_col[:, gs])
                        nc.vector.tensor_sub(
                            t2c[:, gs], en_col[:, gs],
                            b12c[:, 0, gs].bitcast(f32),
                        )
                        nc.vector.tensor_copy(b12c[:, 1, gs], t2c[:, gs])
                        w0 = nc.sync.dma_start(
                            out=eb_v[:, 0, gs], in_=b12c[:, 0, gs].bitcast(f32)
                        )
                        w1 = nc.sync.dma_start(
                            out=eb_v[:, 1, gs], in_=b12c[:, 1, gs].bitcast(f32)
                        )
                        eb_f = wk.tile([2, 512], f32, tag="eb_f",
                                       name=f"eb_f{g}")
                        rd = nc.sync.dma_start(
                            out=eb_f[:],
                            in_=eb_dram.ap()[:, 512 * g:512 * (g + 1)],
                        )
                        # Tile does not track RAW deps through raw DRAM
                        # tensors; order the bounce read after both writes
                        tile.add_dep_helper(rd.ins, w0.ins,
                                            reason="eb bounce RAW")
                        tile.add_dep_helper(rd.ins, w1.ins,
                                            reason="eb bounce RAW")
                        nc.vector.tensor_copy(ebias_q[g][:], eb_f[:])

            # ---------------- main loop ----------------
            with tc.tile_pool(name="ps", bufs=2, space="PSUM") as ps:
                mxall = cp.tile([P, T_TILES, 8], f32, tag="mxall")
                miall = cp.tile([P, T_TILES, 8], u32, tag="miall")
                for t in range(T_TILES):
                    xr = xr_pre.get(t) or load_x(t)

                    pdh = [
                        ps.tile([P, K // 2], f32, tag=f"pd{h}", name=f"pd{h}_{t}")
                        for h in range(2)
                    ]
                    for q in range(4):
                        sl = slice((q % 2) * 512, (q % 2) * 512 + 512)
                        pd = pdh[q // 2]
                        for c in range(4):
                            nc.tensor.matmul(
                                pd[:, sl], xr[:, c], embT2rq[q][:, c],
                                start=(c == 0), stop=False,
                            )
                        # bias last so tile 0 isn't gated on the ebias chain
                        nc.tensor.matmul(
                            pd[:, sl], ones2[:], ebias_q[q][:],
                            start=False, stop=True,
                        )

                    mx = mxall[:, t]
                    mi = miall[:, t]
                    # short PSUM hold: ACT copies scores to SBUF, freeing
                    # the bank pair fast so PE stays warm; DVE ranks SBUF
                    nds = wk.tile([P, K], f32, tag="nds")
                    nc.scalar.activation(nds[:, :K // 2], pdh[0][:], ACT_COPY)
                    nc.scalar.activation(nds[:, K // 2:], pdh[1][:], ACT_COPY)
                    nc.vector.max(out=mx, in_=nds[:])
                    nc.vector.max_index(out=mi, in_max=mx, in_values=nds[:])

                    ft = wk.tile([P, D], f32, tag="ft")
                    nc.gpsimd.indirect_dma_start(
                        out=ft[:],
                        out_offset=None,
                        in_=emb.ap(),
                        in_offset=bass.IndirectOffsetOnAxis(ap=mi[:, :1], axis=0),
                    )
                    nc.sync.dma_start(
                        out=feat.ap()[t * P:(t + 1) * P, :], in_=ft[:]
                    )
                nc.scalar.dma_start(out=vals.ap(), in_=mxall[:])
                nc.scalar.dma_start(out=idxs.ap(), in_=miall[:])

    nc.compile()
    return nc


def _get_program():
    if "nc" not in _cache:
        _cache["nc"] = _build_program()
    return _cache["nc"]


def run_device(flat_pad, emb_np, trace=False):
    """flat_pad: [R_PAD, 512] f32. Returns (feat_pad [R_PAD,512],
    vals [8,13,128,8], idxs [8,13,128,8], results_obj)."""
    from concourse.bass_utils import run_bass_kernel_spmd

    nc = _get_program()
    in_maps = []
    for c in range(N_CORES):
        shard = flat_pad[c * R_CORE:(c + 1) * R_CORE]
        # x5[p, t, c, i] = shard[t*128+i, c*128+p]
        x5 = np.ascontiguousarray(
            shard.reshape(T_TILES, P, 4, P).transpose(3, 0, 2, 1)
        )
        in_maps.append({
            "xT": x5,
            "emb": emb_np,
        })
    out = run_bass_kernel_spmd(
        nc, in_maps, core_ids=list(range(N_CORES)), trace=trace
    )
    res = out.results
    feat_pad = np.concatenate([res[c]["feat"] for c in range(N_CORES)], axis=0)
    vals = np.stack([res[c]["vals"] for c in range(N_CORES)])
    idxs = np.stack([res[c]["idxs"] for c in range(N_CORES)])
    return feat_pad, vals, idxs, out


def kernel(IP_score, emb_weight):
    IP_score = np.asarray(IP_score, dtype=np.float32)
    emb_weight = np.ascontiguousarray(np.asarray(emb_weight, dtype=np.float32))
    flat = IP_score.reshape(N_ROWS, D)
    flat_pad = np.zeros((R_PAD, D), dtype=np.float32)
    flat_pad[:N_ROWS] = flat

    feat_pad, vals, idxs, _ = run_device(flat_pad, emb_weight)

    feature_EMA = feat_pad[:N_ROWS].reshape(IP_score.shape)

    # per-row top value v = max_k (2 x.e - ||e||^2); d = ||x||^2 - v
    # vals[c, p, t, j]: global row = c*1664 + t*128 + p
    v1 = vals[..., 0].transpose(0, 2, 1).reshape(-1)[:N_ROWS].astype(np.float64)
    v3 = vals[..., 2].transpose(0, 2, 1).reshape(-1)[:N_ROWS].astype(np.float64)

    x64 = flat.astype(np.float64)
    sum_rn = float(np.einsum("nd,nd->", x64, x64))
    S1 = sum_rn - float(v1.sum())
    S3 = sum_rn - float(v3.sum())

    k_loss = np.float32(COMMITMENT_COST * S1 / (N_ROWS * D))
    cp_score = np.float32(1.0 - np.sqrt(S1) / np.sqrt(S3))
    return (cp_score, k_loss, feature_EMA)
